# revision 5
# baseline (speedup 1.0000x reference)
"""DiffusionPolicy kernel: CNN backbone + conditioning + radius-masked
transformer encoder + noise head.

Contract: kernel(**inputs) takes the FULL unsharded inputs
(coverage_maps [16,256,4,32,32], actions_t [16,256,2], t [16] int32,
positions [16,256,2], params dict) and returns the FULL output
[16,256,2] float32.

Sharding strategy (data-parallel over B across 8 cores, weights
replicated) is applied when the device path is available; the compute
definition below is exact and self-contained (no imports of problem
files).
"""

import math

import numpy as np

# dims (hardcoded per spec)
B, N, C, H, W = 16, 256, 4, 32, 32
D = 256
HEADS = 8
DH = D // HEADS
LAYERS = 4
FF = 4 * D
LATENT = 32
K = 3
NCNN = 3
RADIUS = 0.25
TIME_DIM = D
NEG = -1e9


def _leaky(x, a=0.01):
    return np.where(x > 0, x, a * x)


def _silu(x):
    # x * sigmoid(x), numerically stable
    return x * (0.5 * (np.tanh(0.5 * x) + 1.0))


def _erf(x):
    # Abramowitz & Stegun 7.1.26 in float64: |err| <= 1.5e-7 (abs)
    x = x.astype(np.float64)
    s = np.sign(x)
    ax = np.abs(x)
    t = 1.0 / (1.0 + 0.3275911 * ax)
    y = 1.0 - (
        ((((1.061405429 * t - 1.453152027) * t) + 1.421413741) * t - 0.284496736) * t
        + 0.254829592
    ) * t * np.exp(-ax * ax)
    return s * y


def _gelu(x):
    return (0.5 * x.astype(np.float64) * (1.0 + _erf(x / math.sqrt(2.0)))).astype(
        np.float32
    )


def _ln(x, g, b, eps=1e-5):
    m = x.mean(-1, keepdims=True)
    v = ((x - m) ** 2).mean(-1, keepdims=True)
    return (x - m) / np.sqrt(v + eps) * g + b


def _mlp2(x, w1, b1, w2, b2):
    return _silu(x @ w1 + b1) @ w2 + b2


def _conv_valid(x, w, b):
    # x: [n, cin, h, w]; w: [cout, cin, k, k]; valid conv, stride 1.
    n, cin, h, ww = x.shape
    cout = w.shape[0]
    ho, wo = h - K + 1, ww - K + 1
    # im2col via stride tricks: patches [n, ho, wo, cin, K, K]
    s = x.strides
    patches = np.lib.stride_tricks.as_strided(
        x,
        shape=(n, cin, ho, wo, K, K),
        strides=(s[0], s[1], s[2], s[3], s[2], s[3]),
        writeable=False,
    )
    # contract (cin, K, K): reorder to [n, ho, wo, cin*K*K]
    col = np.ascontiguousarray(patches.transpose(0, 2, 3, 1, 4, 5)).reshape(
        n * ho * wo, cin * K * K
    )
    wmat = w.reshape(cout, cin * K * K).T  # [cin*K*K, cout]
    out = col @ wmat + b  # [n*ho*wo, cout]
    return out.reshape(n, ho, wo, cout).transpose(0, 3, 1, 2)


def _forward_np(coverage_maps, actions_t, t, positions, p):
    nb = coverage_maps.shape[0]
    imgs = coverage_maps.reshape(nb * N, C, H, W).astype(np.float32)
    feat = np.empty((nb * N, D), np.float32)
    CH = 256  # image chunk to bound im2col memory
    for c0 in range(0, nb * N, CH):
        x = imgs[c0 : c0 + CH]
        for i in range(NCNN):
            x = _conv_valid(x, p[f"conv_w{i}"], np.zeros(LATENT, np.float32))
            x = _leaky(x + p[f"conv_b{i}"][None, :, None, None])
        x = x.reshape(x.shape[0], -1)
        fc = _leaky(x @ p["lin1_w"] + p["lin1_b"])
        feat[c0 : c0 + CH] = fc @ p["proj_w"] + p["proj_b"]
    feat = feat.reshape(nb, N, D)

    a_emb = _mlp2(actions_t, p["act1_w"], p["act1_b"], p["act2_w"], p["act2_b"])
    p_emb = _mlp2(positions, p["pos1_w"], p["pos1_b"], p["pos2_w"], p["pos2_b"])
    half = TIME_DIM // 2
    freqs = np.exp(
        np.linspace(0.0, math.log(10000.0), half, dtype=np.float32)
        * np.float32(-1.0 / (half - 1))
    ).astype(np.float32)
    args = t.astype(np.float32)[:, None] * freqs[None, :]
    t_emb = np.concatenate([np.sin(args), np.cos(args)], -1).astype(np.float32)
    t_emb = _mlp2(t_emb, p["time1_w"], p["time1_b"], p["time2_w"], p["time2_b"])
    tok = feat + a_emb + p_emb + t_emb[:, None, :]

    diff = positions[:, :, None, :] - positions[:, None, :, :]
    allowed = (diff**2).sum(-1) <= RADIUS**2
    mask = np.where(allowed, 0.0, NEG).astype(np.float32)[:, None, :, :]

    x = tok.astype(np.float32)
    for l in range(LAYERS):
        h = _ln(x, p[f"l{l}_ln1_g"], p[f"l{l}_ln1_b"])
        qkv = h @ p[f"l{l}_qkv_w"] + p[f"l{l}_qkv_b"]
        q, k, v = np.split(qkv, 3, axis=-1)
        q = q.reshape(nb, N, HEADS, DH).transpose(0, 2, 1, 3)
        k = k.reshape(nb, N, HEADS, DH).transpose(0, 2, 1, 3)
        v = v.reshape(nb, N, HEADS, DH).transpose(0, 2, 1, 3)
        scores = (
            np.einsum("bhnd,bhmd->bhnm", q, k, optimize=True) / math.sqrt(DH) + mask
        )
        scores = scores - scores.max(-1, keepdims=True)
        e = np.exp(scores)
        attn = e / e.sum(-1, keepdims=True)
        o = np.einsum("bhnm,bhmd->bhnd", attn, v, optimize=True)
        o = o.transpose(0, 2, 1, 3).reshape(nb, N, D)
        x = x + o @ p[f"l{l}_out_w"] + p[f"l{l}_out_b"]
        h = _ln(x, p[f"l{l}_ln2_g"], p[f"l{l}_ln2_b"])
        ff = _gelu(h @ p[f"l{l}_ff1_w"] + p[f"l{l}_ff1_b"])
        x = x + ff @ p[f"l{l}_ff2_w"] + p[f"l{l}_ff2_b"]

    h = _ln(x, p["hd_ln_g"], p["hd_ln_b"])
    return _mlp2(h, p["hd1_w"], p["hd1_b"], p["hd2_w"], p["hd2_b"]).astype(np.float32)


def _make_fwd(jax, jnp, lax):
    """Per-shard forward (batch dim inferred from input). Line-for-line
    the reference network semantics."""

    def fwd(coverage_maps, actions_t, t, positions, p):
        nb = coverage_maps.shape[0]
        x = coverage_maps.reshape(nb * N, C, H, W)
        for i in range(NCNN):
            x = lax.conv_general_dilated(
                x,
                p[f"conv_w{i}"],
                (1, 1),
                "VALID",
                dimension_numbers=("NCHW", "OIHW", "NCHW"),
            )
            x = jax.nn.leaky_relu(x + p[f"conv_b{i}"][None, :, None, None], 0.01)
        x = x.reshape(nb * N, -1)
        feat = jax.nn.leaky_relu(x @ p["lin1_w"] + p["lin1_b"], 0.01)
        feat = (feat @ p["proj_w"] + p["proj_b"]).reshape(nb, N, D)

        def mlp2(x, w1, b1, w2, b2):
            return jax.nn.silu(x @ w1 + b1) @ w2 + b2

        a_emb = mlp2(actions_t, p["act1_w"], p["act1_b"], p["act2_w"], p["act2_b"])
        p_emb = mlp2(positions, p["pos1_w"], p["pos1_b"], p["pos2_w"], p["pos2_b"])
        half = TIME_DIM // 2
        freqs = jnp.exp(
            jnp.linspace(0.0, math.log(10000.0), half) * (-1.0 / (half - 1))
        )
        args = t.astype(jnp.float32)[:, None] * freqs[None, :]
        t_emb = jnp.concatenate([jnp.sin(args), jnp.cos(args)], -1)
        t_emb = mlp2(t_emb, p["time1_w"], p["time1_b"], p["time2_w"], p["time2_b"])
        tok = feat + a_emb + p_emb + t_emb[:, None, :]

        diff = positions[:, :, None, :] - positions[:, None, :, :]
        allowed = (diff**2).sum(-1) <= RADIUS**2
        mask = jnp.where(allowed, 0.0, NEG).astype(jnp.float32)[:, None, :, :]

        def ln(x, g, b, eps=1e-5):
            m = x.mean(-1, keepdims=True)
            v = ((x - m) ** 2).mean(-1, keepdims=True)
            return (x - m) * lax.rsqrt(v + eps) * g + b

        x = tok
        for l in range(LAYERS):
            h = ln(x, p[f"l{l}_ln1_g"], p[f"l{l}_ln1_b"])
            qkv = h @ p[f"l{l}_qkv_w"] + p[f"l{l}_qkv_b"]
            q, k, v = jnp.split(qkv, 3, axis=-1)
            q = q.reshape(nb, N, HEADS, DH).transpose(0, 2, 1, 3)
            k = k.reshape(nb, N, HEADS, DH).transpose(0, 2, 1, 3)
            v = v.reshape(nb, N, HEADS, DH).transpose(0, 2, 1, 3)
            scores = jnp.einsum("bhnd,bhmd->bhnm", q, k) / math.sqrt(DH) + mask
            attn = jax.nn.softmax(scores, axis=-1)
            o = jnp.einsum("bhnm,bhmd->bhnd", attn, v)
            o = o.transpose(0, 2, 1, 3).reshape(nb, N, D)
            x = x + o @ p[f"l{l}_out_w"] + p[f"l{l}_out_b"]
            h = ln(x, p[f"l{l}_ln2_g"], p[f"l{l}_ln2_b"])
            ff = jax.nn.gelu(h @ p[f"l{l}_ff1_w"] + p[f"l{l}_ff1_b"], approximate=False)
            x = x + ff @ p[f"l{l}_ff2_w"] + p[f"l{l}_ff2_b"]

        h = ln(x, p["hd_ln_g"], p["hd_ln_b"])
        return mlp2(h, p["hd1_w"], p["hd1_b"], p["hd2_w"], p["hd2_b"])

    return fwd


def _forward_jax_neuron(coverage_maps, actions_t, t, positions, p):
    """Primary path: 8-way data-parallel over B on the NeuronCores
    (2 envs per core, weights replicated), via the jax/axon backend."""
    import jax
    import jax.numpy as jnp
    from jax import lax

    devs = [d for d in jax.devices() if d.platform != "cpu"][:8]
    assert len(devs) == 8, f"need 8 neuron cores, got {len(devs)}"
    fwd = _make_fwd(jax, jnp, lax)
    M = 8
    shard = lambda a: a.reshape((M, a.shape[0] // M) + a.shape[1:])
    pm = jax.pmap(fwd, in_axes=(0, 0, 0, 0, None), devices=devs)
    out = pm(
        shard(coverage_maps), shard(actions_t), shard(t), shard(positions), p
    )
    out = np.asarray(out)
    return out.reshape(B, N, 2).astype(np.float32)


def _forward_jax(coverage_maps, actions_t, t, positions, p):
    """CPU fallback: same graph, jit on host."""
    import jax
    import jax.numpy as jnp
    from jax import lax

    cpu = jax.devices("cpu")[0]
    fwd = _make_fwd(jax, jnp, lax)
    with jax.default_device(cpu):
        dev_in = jax.device_put((coverage_maps, actions_t, t, positions, p), cpu)
        out = jax.jit(fwd, backend="cpu")(*dev_in)
        return np.asarray(out).astype(np.float32)


def kernel(coverage_maps, actions_t, t, positions, params):
    coverage_maps = np.asarray(coverage_maps, dtype=np.float32)
    actions_t = np.asarray(actions_t, dtype=np.float32)
    t = np.asarray(t)
    positions = np.asarray(positions, dtype=np.float32)
    params = {k: np.asarray(v) for k, v in params.items()}
    try:
        return _forward_jax_neuron(coverage_maps, actions_t, t, positions, params)
    except Exception:
        pass
    try:
        return _forward_jax(coverage_maps, actions_t, t, positions, params)
    except Exception:
        return _forward_np(coverage_maps, actions_t, t, positions, params)
